# revision 10
# baseline (speedup 1.0000x reference)
"""Trainium2 Bass kernel for cascaded double cross-attention.

Reference computation (B=2, N=2048, C=1024, H=16, D=64):
    q = heads(x @ Wq.T); A = heads(x2 @ Wa.T); k, v = heads(x @ Wkv.T)
    ATT_q = softmax(q @ A^T * s);  ATT_k = softmax(A @ k^T * s)
    out = ATT_q @ (ATT_k @ v)

Sharding: 8 cores, core i handles batch b=i//4 and 4 heads g=i%4.

Perf structure on top of the working baseline:
  * Wq/Wa/Wk are pre-scaled by 1/8 on the host, so raw scores arrive as
    u = S*SCALE/8.  exp is then split per tile across TWO engines running
    concurrently: ACT computes exp(8u) exactly (scale=8), and the DVE
    computes (c0+c1*u+c2*u^2)^8 via a custom 8-stage DVE op (EXP8_ANT) --
    a weighted-minimax fit whose softmax-level error washes out (<3e-3
    end-to-end, validated numerically).
  * Score matmuls have K=64 contraction; each is issued as two concurrent
    row-group-tiled matmuls (partitions 0:64 and 64:128) using "swap"
    tiles that mirror each head's kT/aT/qT data into the other partition
    half (built by SBUF-to-SBUF DMA).  This doubles score throughput.
  * AV matmuls, accumulator layout (65-wide slots with a ones column for
    row sums), tails, and DMA staging are as in the baseline.
"""

import sys

if "/opt/trn_rl_repo" not in sys.path:
    sys.path.insert(0, "/opt/trn_rl_repo")

import numpy as np
import ml_dtypes

import concourse.bass as bass
import concourse.tile as tile
from concourse import bacc, mybir
from concourse.bass_utils import run_bass_kernel_spmd

BF16 = ml_dtypes.bfloat16
N_CORES = 8
N, C, H, D = 2048, 1024, 16, 64
HPC = 4  # heads per core
DHC = HPC * D  # 256 output cols per core
CCH = C // 128  # 8 contraction chunks
NB = N // 128  # 16 token blocks
SCALE = float(D) ** -0.5
WPRE = 0.125  # host-side pre-scale of Wq/Wa/Wk; scores arrive as S*SCALE/8
F32 = mybir.dt.float32
BF = mybir.dt.bfloat16
EXP = mybir.ActivationFunctionType.Exp

# exp(8u) ~= (C0 + C1*u + C2*u^2)^8, weighted-minimax fit on u in [-.425,.425]
EXP8_C0 = 1.00009265
EXP8_C1 = 1.00490804
EXP8_C2 = 0.47960157

_CACHE = {}
_EXP8 = None


def _get_exp8_op():
    """Register the EXP8_ANT custom DVE op (idempotent)."""
    global _EXP8
    if _EXP8 is not None:
        return _EXP8
    import concourse.dve_ops as dve_ops_mod
    from concourse.dve_ops import DveOp
    from concourse.dve_spec import Spec, Src0, C0, C1, C2, sq, lower
    from concourse.dve_uop import DveOpSpec

    name = "EXP8_ANT"
    if name in dve_ops_mod._SUB_OPCODE_FOR_NAME:
        _EXP8 = next(op for op in dve_ops_mod.OPS if op.name == name)
        return _EXP8

    spec = Spec(
        body=sq(sq(sq((Src0 * C1 + C0) + sq(Src0) * C2))),
        reference=lambda in0, in1, s0, s1, imm2: (
            (s0 + s1 * in0 + imm2 * in0 * in0) ** 8
        ),
    )
    opcode = dve_ops_mod._CUSTOM_DVE_ROW_BASE + len(dve_ops_mod.OPS)
    shas = {}
    for ver in ("v3", "v4"):
        s = DveOpSpec(name=name, opcode=opcode, uops=lower(spec, ver=ver),
                      rd1_en=False)
        shas[ver] = s.sha(ver)
    op = DveOp(name, spec, subdim=False, uops_sha=shas)
    dve_ops_mod.OPS.append(op)
    dve_ops_mod._SUB_OPCODE_FOR_NAME[name] = opcode
    dve_ops_mod.CUSTOM_DVE_SPECS[name] = spec
    _EXP8 = op
    return op


def _build_program(nreps=1):
    nc = bacc.Bacc("TRN2", target_bir_lowering=False, debug=False,
                   num_devices=N_CORES)

    xt_d = nc.dram_tensor("xt", [C, N], BF, kind="ExternalInput").ap()
    x2t_d = nc.dram_tensor("x2t", [C, N], BF, kind="ExternalInput").ap()
    wq_d = nc.dram_tensor("wq", [C, DHC], BF, kind="ExternalInput").ap()
    wa_d = nc.dram_tensor("wa", [C, DHC], BF, kind="ExternalInput").ap()
    wk_d = nc.dram_tensor("wk", [C, DHC], BF, kind="ExternalInput").ap()
    wv_d = nc.dram_tensor("wv", [C, DHC], BF, kind="ExternalInput").ap()
    out_d = nc.dram_tensor("out", [N, DHC], F32, kind="ExternalOutput").ap()

    with tile.TileContext(nc) as tc:
        for _ in range(nreps):
            _emit(tc, nc, xt_d, x2t_d, wq_d, wa_d, wk_d, wv_d, out_d)
    nc.compile()
    return nc


def _emit(tc, nc, xt_d, x2t_d, wq_d, wa_d, wk_d, wv_d, out_d):
    from contextlib import ExitStack

    exp8 = _get_exp8_op()
    ctx = ExitStack()
    with ctx:
        singles = ctx.enter_context(tc.tile_pool(name="singles", bufs=1))
        ppool = ctx.enter_context(tc.tile_pool(name="ptiles", bufs=5))
        stage_pool = ctx.enter_context(tc.tile_pool(name="stage", bufs=2))
        tmpo_pool = ctx.enter_context(tc.tile_pool(name="tmpones", bufs=2))
        outp = ctx.enter_context(tc.tile_pool(name="outp", bufs=4))
        recp = ctx.enter_context(tc.tile_pool(name="recp", bufs=4))
        psum = ctx.enter_context(
            tc.tile_pool(name="psum", bufs=2, space="PSUM"))
        psum_acc = ctx.enter_context(
            tc.tile_pool(name="psum_acc", bufs=1, space="PSUM"))

        # ---- constants / persistent inputs ----
        xt_sb = singles.tile([128, CCH, N], BF, tag="xt")
        x2t_sb = singles.tile([128, CCH, N], BF, tag="x2t")
        w_sb = {}
        for name in ("wq", "wa", "wk", "wv"):
            w_sb[name] = singles.tile([128, CCH, DHC], BF, tag=name, name=name)
        nc.sync.dma_start(out=w_sb["wk"][:],
                          in_=wk_d.rearrange("(c p) d -> p c d", p=128))
        nc.scalar.dma_start(out=w_sb["wa"][:],
                            in_=wa_d.rearrange("(c p) d -> p c d", p=128))
        nc.gpsimd.dma_start(out=w_sb["wv"][:],
                            in_=wv_d.rearrange("(c p) d -> p c d", p=128))
        nc.gpsimd.dma_start(out=w_sb["wq"][:],
                            in_=wq_d.rearrange("(c p) d -> p c d", p=128))
        xt_r = xt_d.rearrange("(c p) n -> p c n", p=128)
        x2t_r = x2t_d.rearrange("(c p) n -> p c n", p=128)
        for q in range(4):
            ncol = slice(q * 512, (q + 1) * 512)
            nc.sync.dma_start(out=xt_sb[:, :, ncol], in_=xt_r[:, :, ncol])
            nc.scalar.dma_start(out=x2t_sb[:, :, ncol], in_=x2t_r[:, :, ncol])

        # per-pair transposed activations [128, N] bf16: head 2p in
        # partitions 0:64, head 2p+1 in partitions 64:128; plus "swap"
        # tiles with the halves exchanged, so every head's data exists at
        # BOTH partition bases.  Score matmuls then issue as two
        # concurrent row-group-tiled matmuls (rows 0:64 / 64:128).
        qt_p = [singles.tile([128, N], BF, tag=f"qt{p}", name=f"qt{p}")
                for p in range(2)]
        at_p = [singles.tile([128, N], BF, tag=f"at{p}", name=f"at{p}")
                for p in range(2)]
        kt_p = [singles.tile([128, N], BF, tag=f"kt{p}", name=f"kt{p}")
                for p in range(2)]
        qt_s = [singles.tile([128, N], BF, tag=f"qs{p}", name=f"qs{p}")
                for p in range(2)]
        at_s = [singles.tile([128, N], BF, tag=f"as{p}", name=f"as{p}")
                for p in range(2)]
        kt_s = [singles.tile([128, N], BF, tag=f"ks{p}", name=f"ks{p}")
                for p in range(2)]

        def emit_swap(pair_t, swap_t, col0, col1, eng0, eng1):
            """swap tile halves: [head-odd data; head-even data]."""
            cs = slice(col0, col1)
            eng0.dma_start(out=swap_t[0:64, cs], in_=pair_t[64:128, cs])
            eng1.dma_start(out=swap_t[64:128, cs], in_=pair_t[0:64, cs])

        def head_ops(pair_tiles, swap_tiles, h):
            """(lo_view, hi_view): head h's data at partitions 0:64 and
            64:128 respectively."""
            p = h // 2
            if h % 2 == 0:
                return pair_tiles[p][0:64, :], swap_tiles[p][64:128, :]
            return swap_tiles[p][0:64, :], pair_tiles[p][64:128, :]

        # tiny constant used by PSUM-bank-clearing matmuls
        dummy = singles.tile([1, 128], BF, tag="dummy")
        nc.vector.memset(dummy[:], 1.0)

        # staged full output [p, block, head, d] -> one contiguous out DMA
        ot_all = singles.tile([128, NB, HPC, D], F32, tag="ot_all")

        # v in natural layout with a ones column: [j, head, 16, 65]
        v_ones = singles.tile([128, HPC, NB, D + 1], BF, tag="vo")
        nc.vector.memset(v_ones[:, :, :, D:D + 1], 1.0)

        # ---- projections ----
        def emit_pair(name, src_t, pair_tiles, swap_tiles, pair, chunk=1024,
                      copy_eng=None):
            pair_sb = pair_tiles[pair]
            steps = []
            for half0 in range(N // chunk):
                def step(half0=half0):
                    ps = psum.tile([128, chunk], F32,
                                   tag="big" if chunk == 1024 else "fill",
                                   name="ps_p", bufs=2 if chunk == 1024 else 1)
                    for nch in range(chunk // 512):
                        sl = slice(nch * 512, (nch + 1) * 512)
                        gl = slice(half0 * chunk + nch * 512,
                                   half0 * chunk + (nch + 1) * 512)
                        for cc in range(CCH):
                            nc.tensor.matmul(
                                ps[:, sl],
                                lhsT=w_sb[name][:, cc,
                                                pair * 128:(pair + 1) * 128],
                                rhs=src_t[:, cc, gl],
                                start=(cc == 0), stop=(cc == CCH - 1))
                    dst = pair_sb[:, half0 * chunk:(half0 + 1) * chunk]
                    if copy_eng is nc.scalar:
                        nc.scalar.copy(dst, ps[:])
                    else:
                        nc.vector.tensor_copy(dst, ps[:])
                    emit_swap(pair_sb, swap_tiles[pair],
                              half0 * chunk, (half0 + 1) * chunk,
                              nc.sync if half0 % 2 == 0 else nc.gpsimd,
                              nc.scalar if half0 % 2 == 0 else nc.sync)
                steps.append(step)
            return steps

        def emit_v_block(nb):
            pv = psum.tile([128, DHC], F32, tag="fill", name="ps_v",
                            bufs=1)
            for cc in range(CCH):
                nc.tensor.matmul(
                    pv[:, :],
                    lhsT=xt_sb[:, cc, nb * 128:(nb + 1) * 128],
                    rhs=w_sb["wv"][:, cc, :],
                    start=(cc == 0), stop=(cc == CCH - 1))
            # ACT drains v blocks: it is idle during the projection prologue
            # and this keeps the DVE free for the first attention phases
            nc.scalar.copy(
                v_ones[:, :, nb, 0:D],
                pv.rearrange("p (h d) -> p h d", h=HPC))

        ksteps = emit_pair("wk", xt_sb, kt_p, kt_s, 0, copy_eng=nc.scalar)
        asteps = emit_pair("wa", x2t_sb, at_p, at_s, 0, copy_eng=nc.scalar)
        vsteps = [lambda nb=nb: emit_v_block(nb) for nb in range(NB)]
        ksteps[0]()
        asteps[0]()
        for i in range(4):
            vsteps[i]()
        ksteps[1]()
        asteps[1]()
        for i in range(4, NB):
            vsteps[i]()

        # filler queues per attention phase (pair 1 feeds heads 2-3,
        # first needed in phase 4)
        fillers = {
            0: emit_pair("wq", xt_sb, qt_p, qt_s, 0, chunk=512),
            1: emit_pair("wk", xt_sb, kt_p, kt_s, 1, chunk=512),
            2: emit_pair("wa", x2t_sb, at_p, at_s, 1, chunk=512),
            3: emit_pair("wq", xt_sb, qt_p, qt_s, 1, chunk=512),
        }

        # ---- attention per head ----
        LAG = 4
        units = [(blk, half) for blk in range(NB) for half in range(2)]
        pending_tail = None

        def make_score_emitter(lhs_lo, lhs_hi, rhs_lo, rhs_hi):
            def emit_scores(blk, half, use_dve):
                ps = psum.tile([128, 1024], F32, tag="big", name="ps_s")
                bc = slice(blk * 128, (blk + 1) * 128)
                m0 = half * 1024
                nc.tensor.matmul(ps[:, 0:512], lhsT=lhs_lo[:, bc],
                                 rhs=rhs_lo[:, m0:m0 + 512],
                                 start=True, stop=True)
                nc.tensor.matmul(ps[:, 512:1024], lhsT=lhs_hi[:, bc],
                                 rhs=rhs_hi[:, m0 + 512:m0 + 1024],
                                 start=True, stop=True)
                pt = ppool.tile([128, 1024], BF, tag="p", name="pt")
                if use_dve:
                    nc.vector._custom_dve(exp8, out=pt[:], in0=ps[:],
                                          s0=EXP8_C0, s1=EXP8_C1,
                                          imm2=EXP8_C2)
                else:
                    nc.scalar.activation(pt[:], ps[:], EXP, scale=8.0)
                return pt
            return emit_scores

        def acc_off(mb):
            # 7 blocks of 65 fp32 per 512-fp32 PSUM bank (no bank crossing)
            return (mb // 7) * 512 + (mb % 7) * 65

        def emit_av_nat(acc, pt, blk, half):
            """8 AV matmuls: P-tile slices stationary, [v|1]/[tmp|1] moving.
            m-block mb = half*8 + k accumulates at acc_off(mb)."""
            for k in range(8):
                mb = half * 8 + k
                off = acc_off(mb)
                nc.tensor.matmul(
                    acc[:, off:off + D + 1],
                    lhsT=pt[:, k * 128:(k + 1) * 128],
                    rhs=av_rhs[:, blk, :],
                    start=False, stop=(blk == NB - 1),
                    skip_group_check=True)

        BANK_BLKS = [(0, 7), (7, 7), (14, 2)]  # (first block, count) per bank

        def make_tail(acc, dst_tmp_ones, out_head):
            """Bunched tail: reciprocal of row-sum column, normalize, and
            either build [tmp|1] (A1) or stage+DMA the output (A2)."""
            def tail():
                rec = recp.tile([128, NB], F32, tag="rec", name="rec")
                views = []
                for b3, (mb0, nblk) in enumerate(BANK_BLKS):
                    v = acc[:, b3 * 512: b3 * 512 + nblk * 65].rearrange(
                        "p (k c) -> p k c", c=D + 1)
                    views.append((mb0, nblk, v))
                    nc.vector.reciprocal(rec[:, mb0:mb0 + nblk], v[:, :, D])
                if dst_tmp_ones is not None:
                    nc.vector.memset(dst_tmp_ones[:, :, D:D + 1], 1.0)
                    for mb0, nblk, v in views:
                        nc.vector.tensor_tensor(
                            dst_tmp_ones[:, mb0:mb0 + nblk, 0:D],
                            v[:, :, 0:D],
                            rec[:, mb0:mb0 + nblk, None].to_broadcast(
                                (128, nblk, D)),
                            mybir.AluOpType.mult)
                else:
                    for mb0, nblk, v in views:
                        nc.vector.tensor_tensor(
                            ot_all[:, mb0:mb0 + nblk, out_head, :],
                            v[:, :, 0:D],
                            rec[:, mb0:mb0 + nblk, None].to_broadcast(
                                (128, nblk, D)),
                            mybir.AluOpType.mult)
                    if out_head == HPC - 1:
                        out_r = out_d.rearrange("(b p) c -> p b c", p=128)
                        nc.sync.dma_start(out=out_r[:, 0:NB // 2, :],
                                          in_=ot_all[:, 0:NB // 2])
                        nc.scalar.dma_start(out=out_r[:, NB // 2:NB, :],
                                            in_=ot_all[:, NB // 2:NB])
            return tail

        # which units use the DVE exp (vs ACT): alternate so both engines
        # drain score tiles concurrently within every phase; ACT takes a
        # slightly larger share since the DVE also owns tails/copies
        _DVE_MOD16 = {0, 2, 5, 7, 9, 12, 14}  # 14 of 32 units per phase

        def unit_uses_dve(u):
            return (u % 16) in _DVE_MOD16

        tmp_ones = None
        for h in range(HPC):
            for phase in (1, 2):
                if phase == 1:
                    k_lo, k_hi = head_ops(kt_p, kt_s, h)
                    a_lo, a_hi = head_ops(at_p, at_s, h)
                    emit_scores = make_score_emitter(k_lo, k_hi, a_lo, a_hi)
                    av_rhs = v_ones[:, h]
                else:
                    a_lo, a_hi = head_ops(at_p, at_s, h)
                    q_lo, q_hi = head_ops(qt_p, qt_s, h)
                    emit_scores = make_score_emitter(a_lo, a_hi, q_lo, q_hi)
                    av_rhs = tmp_ones
                acc = psum_acc.tile([128, 1536], F32, tag="acc",
                                    name="acc")
                for b3 in range(3):
                    nc.tensor.matmul(
                        acc[:, b3 * 512 + 455: b3 * 512 + 456],
                        lhsT=dummy[:], rhs=dummy[:, 0:1],
                        start=True, stop=True, skip_group_check=True)
                pend = []
                phase_idx = h * 2 + (phase - 1)
                fill = fillers.get(phase_idx, [])
                for u, (blk, half) in enumerate(units):
                    pt = emit_scores(blk, half, unit_uses_dve(u))
                    if pending_tail is not None and u == 0:
                        pending_tail()
                        pending_tail = None
                    if fill:
                        fill.pop(0)()
                    pend.append((pt, blk, half))
                    if len(pend) > LAG:
                        emit_av_nat(acc, *pend.pop(0))
                while fill:
                    fill.pop(0)()
                while pend:
                    emit_av_nat(acc, *pend.pop(0))
                if phase == 1:
                    tmp_ones = tmpo_pool.tile([128, NB, D + 1], BF, tag="to",
                                              name="to")
                    pending_tail = make_tail(acc, tmp_ones, None)
                else:
                    pending_tail = make_tail(acc, None, h)
        pending_tail()


def _get_program(nreps=1):
    key = f"nc{nreps}"
    if key not in _CACHE:
        _CACHE[key] = _build_program(nreps)
    return _CACHE[key]


def _prep_inputs(x, x2, Wq, Wa, Wkv):
    """Host-side shard prep: transpose + cast to bf16 once per batch/group.
    Wq/Wa/Wk are pre-scaled by 1/8 so raw scores equal S*SCALE/8."""
    xt = [np.ascontiguousarray(x[b].T).astype(BF16) for b in range(2)]
    x2t = [np.ascontiguousarray(x2[b].T).astype(BF16) for b in range(2)]
    wq_t = np.ascontiguousarray(Wq.T * WPRE).astype(BF16)     # [C, C]
    wa_t = np.ascontiguousarray(Wa.T * WPRE).astype(BF16)
    wkv_t = np.ascontiguousarray(Wkv.T).astype(BF16)   # [C, 2C]
    wk_t = wkv_t[:, :C].astype(np.float32) * WPRE
    in_maps = []
    for i in range(N_CORES):
        b, g = divmod(i, HPC)
        cols = slice(g * DHC, (g + 1) * DHC)
        in_maps.append({
            "xt": xt[b],
            "x2t": x2t[b],
            "wq": np.ascontiguousarray(wq_t[:, cols]),
            "wa": np.ascontiguousarray(wa_t[:, cols]),
            "wk": np.ascontiguousarray(wk_t[:, cols]).astype(BF16),
            "wv": np.ascontiguousarray(
                wkv_t[:, C + g * DHC: C + (g + 1) * DHC]),
        })
    return in_maps


def kernel(x, x2, Wq, Wa, Wkv, _trace=False, _trace_kwargs=None, _nreps=1):
    _get_exp8_op()
    nc = _get_program(_nreps)
    in_maps = _prep_inputs(
        np.asarray(x, np.float32), np.asarray(x2, np.float32),
        np.asarray(Wq, np.float32), np.asarray(Wa, np.float32),
        np.asarray(Wkv, np.float32))
    res = run_bass_kernel_spmd(nc, in_maps, list(range(N_CORES)),
                               trace=_trace, **(_trace_kwargs or {}))
    out = np.empty((2, N, C), np.float32)
    for i in range(N_CORES):
        b, g = divmod(i, HPC)
        out[b][:, g * DHC:(g + 1) * DHC] = np.asarray(res.results[i]["out"],
                                                      np.float32)
    if _trace:
        return out, res
    return out


# revision 15
# speedup vs baseline: 1.0242x; 1.0242x over previous
"""Trainium2 Bass kernel for cascaded double cross-attention.

Reference computation (B=2, N=2048, C=1024, H=16, D=64):
    q = heads(x @ Wq.T); A = heads(x2 @ Wa.T); k, v = heads(x @ Wkv.T)
    ATT_q = softmax(q @ A^T * s);  ATT_k = softmax(A @ k^T * s)
    out = ATT_q @ (ATT_k @ v)

Sharding: 8 cores, core i handles batch b=i//4 and 4 heads g=i%4.

Perf structure on top of the working baseline:
  * Wq/Wa/Wk are pre-scaled by 1/8 on the host, so raw scores arrive as
    u = S*SCALE/8.  exp is then split per tile across TWO engines running
    concurrently: ACT computes exp(8u) exactly (scale=8), and the DVE
    computes (c0+c1*u+c2*u^2)^8 via a custom 8-stage DVE op (EXP8_ANT) --
    a weighted-minimax fit whose softmax-level error washes out (<3e-3
    end-to-end, validated numerically).
  * Score matmuls have K=64 contraction; each is issued as two concurrent
    row-group-tiled matmuls (partitions 0:64 and 64:128) using "swap"
    tiles that mirror each head's kT/aT/qT data into the other partition
    half (built by SBUF-to-SBUF DMA).  This doubles score throughput.
  * AV matmuls, accumulator layout (65-wide slots with a ones column for
    row sums), tails, and DMA staging are as in the baseline.
"""

import sys

if "/opt/trn_rl_repo" not in sys.path:
    sys.path.insert(0, "/opt/trn_rl_repo")

import numpy as np
import ml_dtypes

import concourse.bass as bass
import concourse.tile as tile
from concourse import bacc, mybir
from concourse.bass_utils import run_bass_kernel_spmd

BF16 = ml_dtypes.bfloat16
N_CORES = 8
N, C, H, D = 2048, 1024, 16, 64
HPC = 4  # heads per core
DHC = HPC * D  # 256 output cols per core
CCH = C // 128  # 8 contraction chunks
NB = N // 128  # 16 token blocks
SCALE = float(D) ** -0.5
WPRE = 0.125  # host-side pre-scale of Wq/Wa/Wk; scores arrive as S*SCALE/8
F32 = mybir.dt.float32
BF = mybir.dt.bfloat16
EXP = mybir.ActivationFunctionType.Exp

# exp(8u) ~= (C0 + C1*u + C2*u^2)^8, weighted-minimax fit on u in [-.425,.425]
EXP8_C0 = 1.00009265
EXP8_C1 = 1.00490804
EXP8_C2 = 0.47960157

_CACHE = {}
_EXP8 = None


def _get_exp8_op():
    """Register the EXP8_ANT custom DVE op (idempotent)."""
    global _EXP8
    if _EXP8 is not None:
        return _EXP8
    import concourse.dve_ops as dve_ops_mod
    from concourse.dve_ops import DveOp
    from concourse.dve_spec import Spec, Src0, C0, C1, C2, sq, lower
    from concourse.dve_uop import DveOpSpec

    name = "EXP8_ANT"
    if name in dve_ops_mod._SUB_OPCODE_FOR_NAME:
        _EXP8 = next(op for op in dve_ops_mod.OPS if op.name == name)
        return _EXP8

    spec = Spec(
        body=sq(sq(sq((Src0 * C1 + C0) + sq(Src0) * C2))),
        reference=lambda in0, in1, s0, s1, imm2: (
            (s0 + s1 * in0 + imm2 * in0 * in0) ** 8
        ),
    )
    opcode = dve_ops_mod._CUSTOM_DVE_ROW_BASE + len(dve_ops_mod.OPS)
    shas = {}
    for ver in ("v3", "v4"):
        s = DveOpSpec(name=name, opcode=opcode, uops=lower(spec, ver=ver),
                      rd1_en=False)
        shas[ver] = s.sha(ver)
    op = DveOp(name, spec, subdim=False, uops_sha=shas)
    dve_ops_mod.OPS.append(op)
    dve_ops_mod._SUB_OPCODE_FOR_NAME[name] = opcode
    dve_ops_mod.CUSTOM_DVE_SPECS[name] = spec
    _EXP8 = op
    return op


def _build_program(nreps=1):
    nc = bacc.Bacc("TRN2", target_bir_lowering=False, debug=False,
                   num_devices=N_CORES)

    xt_d = nc.dram_tensor("xt", [C, N], BF, kind="ExternalInput").ap()
    x2t_d = nc.dram_tensor("x2t", [C, N], BF, kind="ExternalInput").ap()
    wq_d = nc.dram_tensor("wq", [C, DHC], BF, kind="ExternalInput").ap()
    wa_d = nc.dram_tensor("wa", [C, DHC], BF, kind="ExternalInput").ap()
    wk_d = nc.dram_tensor("wk", [C, DHC], BF, kind="ExternalInput").ap()
    wv_d = nc.dram_tensor("wv", [C, DHC], BF, kind="ExternalInput").ap()
    out_d = nc.dram_tensor("out", [N, DHC], F32, kind="ExternalOutput").ap()

    with tile.TileContext(nc) as tc:
        for _ in range(nreps):
            _emit(tc, nc, xt_d, x2t_d, wq_d, wa_d, wk_d, wv_d, out_d)
    nc.compile()
    return nc


def _emit(tc, nc, xt_d, x2t_d, wq_d, wa_d, wk_d, wv_d, out_d):
    from contextlib import ExitStack

    exp8 = _get_exp8_op()
    ctx = ExitStack()
    with ctx:
        singles = ctx.enter_context(tc.tile_pool(name="singles", bufs=1))
        ppool = ctx.enter_context(tc.tile_pool(name="ptiles", bufs=5))
        stage_pool = ctx.enter_context(tc.tile_pool(name="stage", bufs=2))
        tmpo_pool = ctx.enter_context(tc.tile_pool(name="tmpones", bufs=2))
        outp = ctx.enter_context(tc.tile_pool(name="outp", bufs=4))
        recp = ctx.enter_context(tc.tile_pool(name="recp", bufs=4))
        psum = ctx.enter_context(
            tc.tile_pool(name="psum", bufs=2, space="PSUM"))
        psum_acc = ctx.enter_context(
            tc.tile_pool(name="psum_acc", bufs=1, space="PSUM"))

        # ---- constants / persistent inputs ----
        xt_sb = singles.tile([128, CCH, N], BF, tag="xt")
        x2t_sb = singles.tile([128, CCH, N], BF, tag="x2t")
        w_sb = {}
        for name in ("wq", "wa", "wk", "wv"):
            w_sb[name] = singles.tile([128, CCH, DHC], BF, tag=name, name=name)
        nc.sync.dma_start(out=w_sb["wk"][:],
                          in_=wk_d.rearrange("(c p) d -> p c d", p=128))
        nc.scalar.dma_start(out=w_sb["wa"][:],
                            in_=wa_d.rearrange("(c p) d -> p c d", p=128))
        nc.gpsimd.dma_start(out=w_sb["wv"][:],
                            in_=wv_d.rearrange("(c p) d -> p c d", p=128))
        nc.gpsimd.dma_start(out=w_sb["wq"][:],
                            in_=wq_d.rearrange("(c p) d -> p c d", p=128))
        xt_r = xt_d.rearrange("(c p) n -> p c n", p=128)
        x2t_r = x2t_d.rearrange("(c p) n -> p c n", p=128)
        for q in range(4):
            ncol = slice(q * 512, (q + 1) * 512)
            nc.sync.dma_start(out=xt_sb[:, :, ncol], in_=xt_r[:, :, ncol])
            nc.scalar.dma_start(out=x2t_sb[:, :, ncol], in_=x2t_r[:, :, ncol])

        # per-pair transposed activations [128, N] bf16: head 2p in
        # partitions 0:64, head 2p+1 in partitions 64:128; plus "swap"
        # tiles with the halves exchanged, so every head's data exists at
        # BOTH partition bases.  Score matmuls then issue as two
        # concurrent row-group-tiled matmuls (rows 0:64 / 64:128).
        qt_p = [singles.tile([128, N], BF, tag=f"qt{p}", name=f"qt{p}")
                for p in range(2)]
        at_p = [singles.tile([128, N], BF, tag=f"at{p}", name=f"at{p}")
                for p in range(2)]
        kt_p = [singles.tile([128, N], BF, tag=f"kt{p}", name=f"kt{p}")
                for p in range(2)]
        qt_s = [singles.tile([128, N], BF, tag=f"qs{p}", name=f"qs{p}")
                for p in range(2)]
        at_s = [singles.tile([128, N], BF, tag=f"as{p}", name=f"as{p}")
                for p in range(2)]
        kt_s = [singles.tile([128, N], BF, tag=f"ks{p}", name=f"ks{p}")
                for p in range(2)]

        def emit_swap(pair_t, swap_t, col0, col1, eng0, eng1):
            """swap tile halves: [head-odd data; head-even data]."""
            cs = slice(col0, col1)
            eng0.dma_start(out=swap_t[0:64, cs], in_=pair_t[64:128, cs])
            eng1.dma_start(out=swap_t[64:128, cs], in_=pair_t[0:64, cs])

        def head_ops(pair_tiles, swap_tiles, h):
            """(lo_view, hi_view): head h's data at partitions 0:64 and
            64:128 respectively."""
            p = h // 2
            if h % 2 == 0:
                return pair_tiles[p][0:64, :], swap_tiles[p][64:128, :]
            return swap_tiles[p][0:64, :], pair_tiles[p][64:128, :]

        # tiny constant used by PSUM-bank-clearing matmuls
        dummy = singles.tile([1, 128], BF, tag="dummy")
        nc.vector.memset(dummy[:], 1.0)

        # staged full output [p, block, head, d] -> one contiguous out DMA
        ot_all = singles.tile([128, NB, HPC, D], F32, tag="ot_all")

        # v in natural layout with a ones column: [j, head, 16, 65]
        v_ones = singles.tile([128, HPC, NB, D + 1], BF, tag="vo")
        nc.vector.memset(v_ones[:, :, :, D:D + 1], 1.0)

        # ---- projections ----
        def emit_pair(name, src_t, pair_tiles, swap_tiles, pair, chunk=1024,
                      copy_eng=None):
            pair_sb = pair_tiles[pair]
            steps = []
            for half0 in range(N // chunk):
                def step(half0=half0):
                    ps = psum.tile([128, chunk], F32,
                                   tag="big" if chunk == 1024 else "fill",
                                   name="ps_p", bufs=2 if chunk == 1024 else 1)
                    for nch in range(chunk // 512):
                        sl = slice(nch * 512, (nch + 1) * 512)
                        gl = slice(half0 * chunk + nch * 512,
                                   half0 * chunk + (nch + 1) * 512)
                        for cc in range(CCH):
                            nc.tensor.matmul(
                                ps[:, sl],
                                lhsT=w_sb[name][:, cc,
                                                pair * 128:(pair + 1) * 128],
                                rhs=src_t[:, cc, gl],
                                start=(cc == 0), stop=(cc == CCH - 1))
                    dst = pair_sb[:, half0 * chunk:(half0 + 1) * chunk]
                    if copy_eng is nc.scalar:
                        nc.scalar.copy(dst, ps[:])
                    else:
                        nc.vector.tensor_copy(dst, ps[:])
                    emit_swap(pair_sb, swap_tiles[pair],
                              half0 * chunk, (half0 + 1) * chunk,
                              nc.sync if half0 % 2 == 0 else nc.gpsimd,
                              nc.scalar if half0 % 2 == 0 else nc.sync)
                steps.append(step)
            return steps

        def emit_v_block(nb):
            pv = psum.tile([128, DHC], F32, tag="fill", name="ps_v",
                            bufs=1)
            for cc in range(CCH):
                nc.tensor.matmul(
                    pv[:, :],
                    lhsT=xt_sb[:, cc, nb * 128:(nb + 1) * 128],
                    rhs=w_sb["wv"][:, cc, :],
                    start=(cc == 0), stop=(cc == CCH - 1))
            nc.vector.tensor_copy(
                v_ones[:, :, nb, 0:D],
                pv.rearrange("p (h d) -> p h d", h=HPC))

        ksteps = emit_pair("wk", xt_sb, kt_p, kt_s, 0)
        asteps = emit_pair("wa", x2t_sb, at_p, at_s, 0)
        vsteps = [lambda nb=nb: emit_v_block(nb) for nb in range(NB)]
        ksteps[0]()
        asteps[0]()
        for i in range(4):
            vsteps[i]()
        ksteps[1]()
        asteps[1]()
        for i in range(4, NB):
            vsteps[i]()

        # filler queues per attention phase (pair 1 feeds heads 2-3,
        # first needed in phase 4)
        fillers = {
            0: emit_pair("wq", xt_sb, qt_p, qt_s, 0, chunk=512),
            1: emit_pair("wk", xt_sb, kt_p, kt_s, 1, chunk=512),
            2: emit_pair("wa", x2t_sb, at_p, at_s, 1, chunk=512),
            3: emit_pair("wq", xt_sb, qt_p, qt_s, 1, chunk=512),
        }

        # ---- attention per head ----
        LAG = 3
        units = [(blk, half) for blk in range(NB) for half in range(2)]
        pending_tail = None

        def make_score_emitter(lhs_lo, lhs_hi, rhs_lo, rhs_hi):
            def emit_scores(blk, half, use_dve):
                ps = psum.tile([128, 1024], F32, tag="big", name="ps_s")
                bc = slice(blk * 128, (blk + 1) * 128)
                m0 = half * 1024
                nc.tensor.matmul(ps[:, 0:512], lhsT=lhs_lo[:, bc],
                                 rhs=rhs_lo[:, m0:m0 + 512],
                                 start=True, stop=True)
                nc.tensor.matmul(ps[:, 512:1024], lhsT=lhs_hi[:, bc],
                                 rhs=rhs_hi[:, m0 + 512:m0 + 1024],
                                 start=True, stop=True)
                pt = ppool.tile([128, 1024], BF, tag="p", name="pt")
                if use_dve:
                    nc.vector._custom_dve(exp8, out=pt[:], in0=ps[:],
                                          s0=EXP8_C0, s1=EXP8_C1,
                                          imm2=EXP8_C2)
                else:
                    nc.scalar.activation(pt[:], ps[:], EXP, scale=8.0)
                return pt
            return emit_scores

        def acc_off(mb):
            # 7 blocks of 65 fp32 per 512-fp32 PSUM bank (no bank crossing)
            return (mb // 7) * 512 + (mb % 7) * 65

        def emit_av_nat(acc, pt, blk, half):
            """8 AV matmuls: P-tile slices stationary, [v|1]/[tmp|1] moving.
            m-block mb = half*8 + k accumulates at acc_off(mb)."""
            for k in range(8):
                mb = half * 8 + k
                off = acc_off(mb)
                nc.tensor.matmul(
                    acc[:, off:off + D + 1],
                    lhsT=pt[:, k * 128:(k + 1) * 128],
                    rhs=av_rhs[:, blk, :],
                    start=False, stop=(blk == NB - 1),
                    skip_group_check=True)

        BANK_BLKS = [(0, 7), (7, 7), (14, 2)]  # (first block, count) per bank

        def make_tail(acc, dst_tmp_ones, out_head):
            """Bunched tail: reciprocal of row-sum column, normalize, and
            either build [tmp|1] (A1) or stage+DMA the output (A2)."""
            def tail():
                rec = recp.tile([128, NB], F32, tag="rec", name="rec")
                views = []
                for b3, (mb0, nblk) in enumerate(BANK_BLKS):
                    v = acc[:, b3 * 512: b3 * 512 + nblk * 65].rearrange(
                        "p (k c) -> p k c", c=D + 1)
                    views.append((mb0, nblk, v))
                    nc.vector.reciprocal(rec[:, mb0:mb0 + nblk], v[:, :, D])
                if dst_tmp_ones is not None:
                    nc.vector.memset(dst_tmp_ones[:, :, D:D + 1], 1.0)
                    for mb0, nblk, v in views:
                        nc.vector.tensor_tensor(
                            dst_tmp_ones[:, mb0:mb0 + nblk, 0:D],
                            v[:, :, 0:D],
                            rec[:, mb0:mb0 + nblk, None].to_broadcast(
                                (128, nblk, D)),
                            mybir.AluOpType.mult)
                else:
                    for mb0, nblk, v in views:
                        nc.vector.tensor_tensor(
                            ot_all[:, mb0:mb0 + nblk, out_head, :],
                            v[:, :, 0:D],
                            rec[:, mb0:mb0 + nblk, None].to_broadcast(
                                (128, nblk, D)),
                            mybir.AluOpType.mult)
                    # stream this head's columns out now: overlaps the DMA
                    # with the remaining heads' compute instead of one
                    # exposed burst after the last tail
                    out_r = out_d.rearrange("(b p) (h d) -> p b h d",
                                            p=128, h=HPC)
                    hh = out_head
                    nc.sync.dma_start(
                        out=out_r[:, 0:NB // 2, hh, :],
                        in_=ot_all[:, 0:NB // 2, hh, :])
                    nc.scalar.dma_start(
                        out=out_r[:, NB // 2:NB, hh, :],
                        in_=ot_all[:, NB // 2:NB, hh, :])
            return tail

        # which units use the DVE exp (vs ACT): strict alternation so both
        # engines drain score tiles concurrently within every phase (the
        # 2-deep score ring gives each engine a dedicated bank pair)
        def unit_uses_dve(u):
            return u % 2 == 0

        tmp_ones = None
        for h in range(HPC):
            for phase in (1, 2):
                if phase == 1:
                    k_lo, k_hi = head_ops(kt_p, kt_s, h)
                    a_lo, a_hi = head_ops(at_p, at_s, h)
                    emit_scores = make_score_emitter(k_lo, k_hi, a_lo, a_hi)
                    av_rhs = v_ones[:, h]
                else:
                    a_lo, a_hi = head_ops(at_p, at_s, h)
                    q_lo, q_hi = head_ops(qt_p, qt_s, h)
                    emit_scores = make_score_emitter(a_lo, a_hi, q_lo, q_hi)
                    av_rhs = tmp_ones
                acc = psum_acc.tile([128, 1536], F32, tag="acc",
                                    name="acc")
                for b3 in range(3):
                    nc.tensor.matmul(
                        acc[:, b3 * 512 + 455: b3 * 512 + 456],
                        lhsT=dummy[:], rhs=dummy[:, 0:1],
                        start=True, stop=True, skip_group_check=True)
                pend = []
                phase_idx = h * 2 + (phase - 1)
                fill = fillers.get(phase_idx, [])
                for u, (blk, half) in enumerate(units):
                    pt = emit_scores(blk, half, unit_uses_dve(u))
                    if pending_tail is not None and u == 0:
                        pending_tail()
                        pending_tail = None
                    if fill:
                        fill.pop(0)()
                    pend.append((pt, blk, half))
                    if len(pend) > LAG:
                        emit_av_nat(acc, *pend.pop(0))
                while fill:
                    fill.pop(0)()
                while pend:
                    emit_av_nat(acc, *pend.pop(0))
                if phase == 1:
                    tmp_ones = tmpo_pool.tile([128, NB, D + 1], BF, tag="to",
                                              name="to")
                    pending_tail = make_tail(acc, tmp_ones, None)
                else:
                    pending_tail = make_tail(acc, None, h)
        pending_tail()


def _get_program(nreps=1):
    key = f"nc{nreps}"
    if key not in _CACHE:
        _CACHE[key] = _build_program(nreps)
    return _CACHE[key]


def _prep_inputs(x, x2, Wq, Wa, Wkv):
    """Host-side shard prep: transpose + cast to bf16 once per batch/group.
    Wq/Wa/Wk are pre-scaled by 1/8 so raw scores equal S*SCALE/8."""
    xt = [np.ascontiguousarray(x[b].T).astype(BF16) for b in range(2)]
    x2t = [np.ascontiguousarray(x2[b].T).astype(BF16) for b in range(2)]
    wq_t = np.ascontiguousarray(Wq.T * WPRE).astype(BF16)     # [C, C]
    wa_t = np.ascontiguousarray(Wa.T * WPRE).astype(BF16)
    wkv_t = np.ascontiguousarray(Wkv.T).astype(BF16)   # [C, 2C]
    wk_t = wkv_t[:, :C].astype(np.float32) * WPRE
    in_maps = []
    for i in range(N_CORES):
        b, g = divmod(i, HPC)
        cols = slice(g * DHC, (g + 1) * DHC)
        in_maps.append({
            "xt": xt[b],
            "x2t": x2t[b],
            "wq": np.ascontiguousarray(wq_t[:, cols]),
            "wa": np.ascontiguousarray(wa_t[:, cols]),
            "wk": np.ascontiguousarray(wk_t[:, cols]).astype(BF16),
            "wv": np.ascontiguousarray(
                wkv_t[:, C + g * DHC: C + (g + 1) * DHC]),
        })
    return in_maps


def kernel(x, x2, Wq, Wa, Wkv, _trace=False, _trace_kwargs=None, _nreps=1):
    _get_exp8_op()
    nc = _get_program(_nreps)
    in_maps = _prep_inputs(
        np.asarray(x, np.float32), np.asarray(x2, np.float32),
        np.asarray(Wq, np.float32), np.asarray(Wa, np.float32),
        np.asarray(Wkv, np.float32))
    res = run_bass_kernel_spmd(nc, in_maps, list(range(N_CORES)),
                               trace=_trace, **(_trace_kwargs or {}))
    out = np.empty((2, N, C), np.float32)
    for i in range(N_CORES):
        b, g = divmod(i, HPC)
        out[b][:, g * DHC:(g + 1) * DHC] = np.asarray(res.results[i]["out"],
                                                      np.float32)
    if _trace:
        return out, res
    return out


# revision 16
# speedup vs baseline: 1.0249x; 1.0006x over previous
"""Trainium2 Bass kernel for cascaded double cross-attention.

Reference computation (B=2, N=2048, C=1024, H=16, D=64):
    q = heads(x @ Wq.T); A = heads(x2 @ Wa.T); k, v = heads(x @ Wkv.T)
    ATT_q = softmax(q @ A^T * s);  ATT_k = softmax(A @ k^T * s)
    out = ATT_q @ (ATT_k @ v)

Sharding: 8 cores, core i handles batch b=i//4 and 4 heads g=i%4.

Perf structure on top of the working baseline:
  * Wq/Wa/Wk are pre-scaled by 1/8 on the host, so raw scores arrive as
    u = S*SCALE/8.  exp is then split per tile across TWO engines running
    concurrently: ACT computes exp(8u) exactly (scale=8), and the DVE
    computes (c0+c1*u+c2*u^2)^8 via a custom 8-stage DVE op (EXP8_ANT) --
    a weighted-minimax fit whose softmax-level error washes out (<3e-3
    end-to-end, validated numerically).
  * Score matmuls have K=64 contraction; each is issued as two concurrent
    row-group-tiled matmuls (partitions 0:64 and 64:128) using "swap"
    tiles that mirror each head's kT/aT/qT data into the other partition
    half (built by SBUF-to-SBUF DMA).  This doubles score throughput.
  * AV matmuls, accumulator layout (65-wide slots with a ones column for
    row sums), tails, and DMA staging are as in the baseline.
"""

import sys

if "/opt/trn_rl_repo" not in sys.path:
    sys.path.insert(0, "/opt/trn_rl_repo")

import numpy as np
import ml_dtypes

import concourse.bass as bass
import concourse.tile as tile
from concourse import bacc, mybir
from concourse.bass_utils import run_bass_kernel_spmd

BF16 = ml_dtypes.bfloat16
N_CORES = 8
N, C, H, D = 2048, 1024, 16, 64
HPC = 4  # heads per core
DHC = HPC * D  # 256 output cols per core
CCH = C // 128  # 8 contraction chunks
NB = N // 128  # 16 token blocks
SCALE = float(D) ** -0.5
WPRE = 0.125  # host-side pre-scale of Wq/Wa/Wk; scores arrive as S*SCALE/8
F32 = mybir.dt.float32
BF = mybir.dt.bfloat16
EXP = mybir.ActivationFunctionType.Exp

# exp(8u) ~= (C0 + C1*u + C2*u^2)^8, weighted-minimax fit on u in [-.425,.425]
EXP8_C0 = 1.00009265
EXP8_C1 = 1.00490804
EXP8_C2 = 0.47960157

_CACHE = {}
_EXP8 = None


def _get_exp8_op():
    """Register the EXP8_ANT custom DVE op (idempotent)."""
    global _EXP8
    if _EXP8 is not None:
        return _EXP8
    import concourse.dve_ops as dve_ops_mod
    from concourse.dve_ops import DveOp
    from concourse.dve_spec import Spec, Src0, C0, C1, C2, sq, lower
    from concourse.dve_uop import DveOpSpec

    name = "EXP8_ANT"
    if name in dve_ops_mod._SUB_OPCODE_FOR_NAME:
        _EXP8 = next(op for op in dve_ops_mod.OPS if op.name == name)
        return _EXP8

    spec = Spec(
        body=sq(sq(sq((Src0 * C1 + C0) + sq(Src0) * C2))),
        reference=lambda in0, in1, s0, s1, imm2: (
            (s0 + s1 * in0 + imm2 * in0 * in0) ** 8
        ),
    )
    opcode = dve_ops_mod._CUSTOM_DVE_ROW_BASE + len(dve_ops_mod.OPS)
    shas = {}
    for ver in ("v3", "v4"):
        s = DveOpSpec(name=name, opcode=opcode, uops=lower(spec, ver=ver),
                      rd1_en=False)
        shas[ver] = s.sha(ver)
    op = DveOp(name, spec, subdim=False, uops_sha=shas)
    dve_ops_mod.OPS.append(op)
    dve_ops_mod._SUB_OPCODE_FOR_NAME[name] = opcode
    dve_ops_mod.CUSTOM_DVE_SPECS[name] = spec
    _EXP8 = op
    return op


def _build_program(nreps=1):
    nc = bacc.Bacc("TRN2", target_bir_lowering=False, debug=False,
                   num_devices=N_CORES)

    xt_d = nc.dram_tensor("xt", [C, N], BF, kind="ExternalInput").ap()
    x2t_d = nc.dram_tensor("x2t", [C, N], BF, kind="ExternalInput").ap()
    wq_d = nc.dram_tensor("wq", [C, DHC], BF, kind="ExternalInput").ap()
    wa_d = nc.dram_tensor("wa", [C, DHC], BF, kind="ExternalInput").ap()
    wk_d = nc.dram_tensor("wk", [C, DHC], BF, kind="ExternalInput").ap()
    wv_d = nc.dram_tensor("wv", [C, DHC], BF, kind="ExternalInput").ap()
    out_d = nc.dram_tensor("out", [N, DHC], F32, kind="ExternalOutput").ap()

    with tile.TileContext(nc) as tc:
        for _ in range(nreps):
            _emit(tc, nc, xt_d, x2t_d, wq_d, wa_d, wk_d, wv_d, out_d)
    nc.compile()
    return nc


def _emit(tc, nc, xt_d, x2t_d, wq_d, wa_d, wk_d, wv_d, out_d):
    from contextlib import ExitStack

    exp8 = _get_exp8_op()
    ctx = ExitStack()
    with ctx:
        singles = ctx.enter_context(tc.tile_pool(name="singles", bufs=1))
        ppool = ctx.enter_context(tc.tile_pool(name="ptiles", bufs=5))
        stage_pool = ctx.enter_context(tc.tile_pool(name="stage", bufs=2))
        tmpo_pool = ctx.enter_context(tc.tile_pool(name="tmpones", bufs=2))
        outp = ctx.enter_context(tc.tile_pool(name="outp", bufs=4))
        recp = ctx.enter_context(tc.tile_pool(name="recp", bufs=4))
        psum = ctx.enter_context(
            tc.tile_pool(name="psum", bufs=2, space="PSUM"))
        psum_acc = ctx.enter_context(
            tc.tile_pool(name="psum_acc", bufs=1, space="PSUM"))

        # ---- constants / persistent inputs ----
        xt_sb = singles.tile([128, CCH, N], BF, tag="xt")
        x2t_sb = singles.tile([128, CCH, N], BF, tag="x2t")
        w_sb = {}
        for name in ("wq", "wa", "wk", "wv"):
            w_sb[name] = singles.tile([128, CCH, DHC], BF, tag=name, name=name)
        nc.sync.dma_start(out=w_sb["wk"][:],
                          in_=wk_d.rearrange("(c p) d -> p c d", p=128))
        nc.scalar.dma_start(out=w_sb["wa"][:],
                            in_=wa_d.rearrange("(c p) d -> p c d", p=128))
        nc.gpsimd.dma_start(out=w_sb["wv"][:],
                            in_=wv_d.rearrange("(c p) d -> p c d", p=128))
        nc.gpsimd.dma_start(out=w_sb["wq"][:],
                            in_=wq_d.rearrange("(c p) d -> p c d", p=128))
        xt_r = xt_d.rearrange("(c p) n -> p c n", p=128)
        x2t_r = x2t_d.rearrange("(c p) n -> p c n", p=128)
        for q in range(4):
            ncol = slice(q * 512, (q + 1) * 512)
            nc.sync.dma_start(out=xt_sb[:, :, ncol], in_=xt_r[:, :, ncol])
            nc.scalar.dma_start(out=x2t_sb[:, :, ncol], in_=x2t_r[:, :, ncol])

        # per-pair transposed activations [128, N] bf16: head 2p in
        # partitions 0:64, head 2p+1 in partitions 64:128; plus "swap"
        # tiles with the halves exchanged, so every head's data exists at
        # BOTH partition bases.  Score matmuls then issue as two
        # concurrent row-group-tiled matmuls (rows 0:64 / 64:128).
        qt_p = [singles.tile([128, N], BF, tag=f"qt{p}", name=f"qt{p}")
                for p in range(2)]
        at_p = [singles.tile([128, N], BF, tag=f"at{p}", name=f"at{p}")
                for p in range(2)]
        kt_p = [singles.tile([128, N], BF, tag=f"kt{p}", name=f"kt{p}")
                for p in range(2)]
        qt_s = [singles.tile([128, N], BF, tag=f"qs{p}", name=f"qs{p}")
                for p in range(2)]
        at_s = [singles.tile([128, N], BF, tag=f"as{p}", name=f"as{p}")
                for p in range(2)]
        kt_s = [singles.tile([128, N], BF, tag=f"ks{p}", name=f"ks{p}")
                for p in range(2)]

        def emit_swap(pair_t, swap_t, col0, col1, eng0, eng1):
            """swap tile halves: [head-odd data; head-even data]."""
            cs = slice(col0, col1)
            eng0.dma_start(out=swap_t[0:64, cs], in_=pair_t[64:128, cs])
            eng1.dma_start(out=swap_t[64:128, cs], in_=pair_t[0:64, cs])

        def head_ops(pair_tiles, swap_tiles, h):
            """(lo_view, hi_view): head h's data at partitions 0:64 and
            64:128 respectively."""
            p = h // 2
            if h % 2 == 0:
                return pair_tiles[p][0:64, :], swap_tiles[p][64:128, :]
            return swap_tiles[p][0:64, :], pair_tiles[p][64:128, :]

        # tiny constant used by PSUM-bank-clearing matmuls
        dummy = singles.tile([1, 128], BF, tag="dummy")
        nc.vector.memset(dummy[:], 1.0)

        # staged full output [p, block, head, d] -> one contiguous out DMA
        ot_all = singles.tile([128, NB, HPC, D], F32, tag="ot_all")

        # v in natural layout with a ones column: [j, head, 16, 65]
        v_ones = singles.tile([128, HPC, NB, D + 1], BF, tag="vo")
        nc.vector.memset(v_ones[:, :, :, D:D + 1], 1.0)

        # ---- projections ----
        def emit_pair(name, src_t, pair_tiles, swap_tiles, pair, chunk=1024,
                      copy_eng=None):
            pair_sb = pair_tiles[pair]
            steps = []
            for half0 in range(N // chunk):
                def step(half0=half0):
                    ps = psum.tile([128, chunk], F32,
                                   tag="big" if chunk == 1024 else "fill",
                                   name="ps_p", bufs=2 if chunk == 1024 else 1)
                    for nch in range(chunk // 512):
                        sl = slice(nch * 512, (nch + 1) * 512)
                        gl = slice(half0 * chunk + nch * 512,
                                   half0 * chunk + (nch + 1) * 512)
                        for cc in range(CCH):
                            nc.tensor.matmul(
                                ps[:, sl],
                                lhsT=w_sb[name][:, cc,
                                                pair * 128:(pair + 1) * 128],
                                rhs=src_t[:, cc, gl],
                                start=(cc == 0), stop=(cc == CCH - 1))
                    dst = pair_sb[:, half0 * chunk:(half0 + 1) * chunk]
                    if copy_eng is nc.scalar:
                        nc.scalar.copy(dst, ps[:])
                    else:
                        nc.vector.tensor_copy(dst, ps[:])
                    emit_swap(pair_sb, swap_tiles[pair],
                              half0 * chunk, (half0 + 1) * chunk,
                              nc.sync if half0 % 2 == 0 else nc.gpsimd,
                              nc.scalar if half0 % 2 == 0 else nc.sync)
                steps.append(step)
            return steps

        def emit_v_block(nb):
            pv = psum.tile([128, DHC], F32, tag="fill", name="ps_v",
                            bufs=1)
            for cc in range(CCH):
                nc.tensor.matmul(
                    pv[:, :],
                    lhsT=xt_sb[:, cc, nb * 128:(nb + 1) * 128],
                    rhs=w_sb["wv"][:, cc, :],
                    start=(cc == 0), stop=(cc == CCH - 1))
            nc.vector.tensor_copy(
                v_ones[:, :, nb, 0:D],
                pv.rearrange("p (h d) -> p h d", h=HPC))

        ksteps = emit_pair("wk", xt_sb, kt_p, kt_s, 0)
        asteps = emit_pair("wa", x2t_sb, at_p, at_s, 0)
        vsteps = [lambda nb=nb: emit_v_block(nb) for nb in range(NB)]
        ksteps[0]()
        asteps[0]()
        for i in range(4):
            vsteps[i]()
        ksteps[1]()
        asteps[1]()
        for i in range(4, NB):
            vsteps[i]()

        # filler queues per attention phase (pair 1 feeds heads 2-3,
        # first needed in phase 4)
        fillers = {
            0: emit_pair("wq", xt_sb, qt_p, qt_s, 0, chunk=512),
            1: emit_pair("wk", xt_sb, kt_p, kt_s, 1, chunk=512),
            2: emit_pair("wa", x2t_sb, at_p, at_s, 1, chunk=512),
            3: emit_pair("wq", xt_sb, qt_p, qt_s, 1, chunk=512),
        }

        # ---- attention per head ----
        LAG = 3
        units = [(blk, half) for blk in range(NB) for half in range(2)]
        pending_tail = None

        def make_score_emitter(lhs_lo, lhs_hi, rhs_lo, rhs_hi):
            def emit_scores(blk, half, use_dve):
                ps = psum.tile([128, 1024], F32, tag="big", name="ps_s")
                bc = slice(blk * 128, (blk + 1) * 128)
                m0 = half * 1024
                nc.tensor.matmul(ps[:, 0:512], lhsT=lhs_lo[:, bc],
                                 rhs=rhs_lo[:, m0:m0 + 512],
                                 start=True, stop=True)
                nc.tensor.matmul(ps[:, 512:1024], lhsT=lhs_hi[:, bc],
                                 rhs=rhs_hi[:, m0 + 512:m0 + 1024],
                                 start=True, stop=True)
                pt = ppool.tile([128, 1024], BF, tag="p", name="pt")
                if use_dve:
                    nc.vector._custom_dve(exp8, out=pt[:], in0=ps[:],
                                          s0=EXP8_C0, s1=EXP8_C1,
                                          imm2=EXP8_C2)
                else:
                    nc.scalar.activation(pt[:], ps[:], EXP, scale=8.0)
                return pt
            return emit_scores

        def acc_off(mb):
            # 7 blocks of 65 fp32 per 512-fp32 PSUM bank (no bank crossing)
            return (mb // 7) * 512 + (mb % 7) * 65

        def emit_av_nat(acc, pt, blk, half):
            """8 AV matmuls: P-tile slices stationary, [v|1]/[tmp|1] moving.
            m-block mb = half*8 + k accumulates at acc_off(mb)."""
            for k in range(8):
                mb = half * 8 + k
                off = acc_off(mb)
                nc.tensor.matmul(
                    acc[:, off:off + D + 1],
                    lhsT=pt[:, k * 128:(k + 1) * 128],
                    rhs=av_rhs[:, blk, :],
                    start=False, stop=(blk == NB - 1),
                    skip_group_check=True)

        BANK_BLKS = [(0, 7), (7, 7), (14, 2)]  # (first block, count) per bank

        def make_tail(acc, dst_tmp_ones, out_head):
            """Bunched tail: reciprocal of row-sum column, normalize, and
            either build [tmp|1] (A1) or stage+DMA the output (A2)."""
            def tail():
                rec = recp.tile([128, NB], F32, tag="rec", name="rec")
                views = []
                for b3, (mb0, nblk) in enumerate(BANK_BLKS):
                    v = acc[:, b3 * 512: b3 * 512 + nblk * 65].rearrange(
                        "p (k c) -> p k c", c=D + 1)
                    views.append((mb0, nblk, v))
                    nc.vector.reciprocal(rec[:, mb0:mb0 + nblk], v[:, :, D])
                if dst_tmp_ones is not None:
                    nc.vector.memset(dst_tmp_ones[:, :, D:D + 1], 1.0)
                    for mb0, nblk, v in views:
                        nc.vector.tensor_tensor(
                            dst_tmp_ones[:, mb0:mb0 + nblk, 0:D],
                            v[:, :, 0:D],
                            rec[:, mb0:mb0 + nblk, None].to_broadcast(
                                (128, nblk, D)),
                            mybir.AluOpType.mult)
                else:
                    for mb0, nblk, v in views:
                        nc.vector.tensor_tensor(
                            ot_all[:, mb0:mb0 + nblk, out_head, :],
                            v[:, :, 0:D],
                            rec[:, mb0:mb0 + nblk, None].to_broadcast(
                                (128, nblk, D)),
                            mybir.AluOpType.mult)
                    if out_head == HPC - 1:
                        out_r = out_d.rearrange("(b p) c -> p b c", p=128)
                        nc.sync.dma_start(out=out_r[:, 0:NB // 2, :],
                                          in_=ot_all[:, 0:NB // 2])
                        nc.scalar.dma_start(out=out_r[:, NB // 2:NB, :],
                                            in_=ot_all[:, NB // 2:NB])
            return tail

        # which units use the DVE exp (vs ACT): strict alternation so both
        # engines drain score tiles concurrently within every phase (the
        # 2-deep score ring gives each engine a dedicated bank pair)
        def unit_uses_dve(u):
            return u % 2 == 0

        tmp_ones = None
        for h in range(HPC):
            for phase in (1, 2):
                if phase == 1:
                    k_lo, k_hi = head_ops(kt_p, kt_s, h)
                    a_lo, a_hi = head_ops(at_p, at_s, h)
                    emit_scores = make_score_emitter(k_lo, k_hi, a_lo, a_hi)
                    av_rhs = v_ones[:, h]
                else:
                    a_lo, a_hi = head_ops(at_p, at_s, h)
                    q_lo, q_hi = head_ops(qt_p, qt_s, h)
                    emit_scores = make_score_emitter(a_lo, a_hi, q_lo, q_hi)
                    av_rhs = tmp_ones
                acc = psum_acc.tile([128, 1536], F32, tag="acc",
                                    name="acc")
                for b3 in range(3):
                    nc.tensor.matmul(
                        acc[:, b3 * 512 + 455: b3 * 512 + 456],
                        lhsT=dummy[:], rhs=dummy[:, 0:1],
                        start=True, stop=True, skip_group_check=True)
                pend = []
                phase_idx = h * 2 + (phase - 1)
                fill = fillers.get(phase_idx, [])
                for u, (blk, half) in enumerate(units):
                    pt = emit_scores(blk, half, unit_uses_dve(u))
                    if pending_tail is not None and u == 0:
                        pending_tail()
                        pending_tail = None
                    if fill:
                        fill.pop(0)()
                    pend.append((pt, blk, half))
                    if len(pend) > LAG:
                        emit_av_nat(acc, *pend.pop(0))
                while fill:
                    fill.pop(0)()
                while pend:
                    emit_av_nat(acc, *pend.pop(0))
                if phase == 1:
                    tmp_ones = tmpo_pool.tile([128, NB, D + 1], BF, tag="to",
                                              name="to")
                    pending_tail = make_tail(acc, tmp_ones, None)
                else:
                    pending_tail = make_tail(acc, None, h)
        pending_tail()


def _get_program(nreps=1):
    key = f"nc{nreps}"
    if key not in _CACHE:
        _CACHE[key] = _build_program(nreps)
    return _CACHE[key]


def _prep_inputs(x, x2, Wq, Wa, Wkv):
    """Host-side shard prep: transpose + cast to bf16 once per batch/group.
    Wq/Wa/Wk are pre-scaled by 1/8 so raw scores equal S*SCALE/8."""
    xt = [np.ascontiguousarray(x[b].T).astype(BF16) for b in range(2)]
    x2t = [np.ascontiguousarray(x2[b].T).astype(BF16) for b in range(2)]
    wq_t = np.ascontiguousarray(Wq.T * WPRE).astype(BF16)     # [C, C]
    wa_t = np.ascontiguousarray(Wa.T * WPRE).astype(BF16)
    wkv_t = np.ascontiguousarray(Wkv.T).astype(BF16)   # [C, 2C]
    wk_t = wkv_t[:, :C].astype(np.float32) * WPRE
    in_maps = []
    for i in range(N_CORES):
        b, g = divmod(i, HPC)
        cols = slice(g * DHC, (g + 1) * DHC)
        in_maps.append({
            "xt": xt[b],
            "x2t": x2t[b],
            "wq": np.ascontiguousarray(wq_t[:, cols]),
            "wa": np.ascontiguousarray(wa_t[:, cols]),
            "wk": np.ascontiguousarray(wk_t[:, cols]).astype(BF16),
            "wv": np.ascontiguousarray(
                wkv_t[:, C + g * DHC: C + (g + 1) * DHC]),
        })
    return in_maps


def kernel(x, x2, Wq, Wa, Wkv, _trace=False, _trace_kwargs=None, _nreps=1):
    _get_exp8_op()
    nc = _get_program(_nreps)
    in_maps = _prep_inputs(
        np.asarray(x, np.float32), np.asarray(x2, np.float32),
        np.asarray(Wq, np.float32), np.asarray(Wa, np.float32),
        np.asarray(Wkv, np.float32))
    res = run_bass_kernel_spmd(nc, in_maps, list(range(N_CORES)),
                               trace=_trace, **(_trace_kwargs or {}))
    out = np.empty((2, N, C), np.float32)
    for i in range(N_CORES):
        b, g = divmod(i, HPC)
        out[b][:, g * DHC:(g + 1) * DHC] = np.asarray(res.results[i]["out"],
                                                      np.float32)
    if _trace:
        return out, res
    return out
